# revision 16
# baseline (speedup 1.0000x reference)
"""Trainium2 Bass kernel for nn_Column (nms_detection).

Computation (matches the reference exactly):
  out[t,k]  = sum_chw rec_field[t,chw] * weight[k,chw]        (32x512 <- contract 100000)
  pot       = out * (out > 10) ; spike = (out > 10)
  nspk[k]   = sum_t spike ; first[k] = min(32 - nspk, 31)
  values[k] = pot[first[k], k] ; v = max_k(values) * 32
  total     = nspk*values + nspk*v
  coef      = top-16 nonzero mask of total (== sequential argmax-suppress set)
  result    = spike * coef[broadcast]                          (32x512 of 0.0/1.0)

Distribution: contraction dim (100000) sharded 8 ways (12500 rows/core, padded
to 12544 = 98*128).  Per chunk of 128 contraction rows the stationary is
[x_hi | x_lo] (128,64) bf16; the moving operand is [w_hi | w_lo]: both passes
accumulate x*(w_hi+w_lo) into one PSUM bank, which with the hi/lo x rows gives
full fp32 precision.  The weight stream (25.7MB/core) is feature-major over
two splits (384/128 features) on both HWDGE queues (~7KB/partition
descriptors, ~320GB/s); x rides SWDGE.

Each split's partial is folded into packed rows of S (row 32q+t, col c <->
feature 128q+c) and AllReduced separately: AR0 (48KB) is triggered while
split 1 still streams, so mostly only AR1 (16KB) is exposed.  The collectives
engine's ~50us bootstrap barrier runs concurrently with the stream.  (A
remote-DMA broadcast exchange was tried instead: it is descriptor-latency
bound at ~68us -- a 128-partition remote send is 128 serial ~1us cross-die
descriptors -- and without any collective in the NEFF the runtime does not
synchronize core launches, so remote-sem waits absorb multi-ms skew.)

Every core runs the k-WTA epilogue redundantly in the packed layout (nspk via
a block-diagonal ones matmul, values via per-block extraction matmuls, top-16
via two 8-max/match-replace rounds); split-0 epilogue prework hides under AR1.
Core 0's (128,128) packed result is unpacked on host.
"""

import numpy as np
import ml_dtypes

import concourse.bacc as bacc
import concourse.mybir as mybir
from concourse.tile import TileContext
from concourse.bass_utils import run_bass_kernel_spmd

T = 32               # timesteps
K = 512              # out_channels / features
CTOT = 100000        # in_channels * rf_size * length (1*50*2000)
NCORES = 8
SH = CTOT // NCORES  # 12500 contraction rows per core
NCH = 98             # 128-row contraction chunks per core
SHP = NCH * 128      # 12544 (zero padded)
GROUP = 4            # chunks per x head-start piece
SPLITS = [(0, 384), (384, 512)]  # feature splits: AR0 hides under split-1
GROUPS = [5, 14]      # chunks per W DMA group per split (~7KB/partition descs)
QPAR = [0, 1]         # first queue (0=sync) per split, balances queue bytes
THRESH = 10.0
F32 = mybir.dt.float32
BF16 = mybir.dt.bfloat16
NPBF16 = ml_dtypes.bfloat16

_CACHE = {}


def _build_nc():
    nc = bacc.Bacc("TRN2", target_bir_lowering=False, debug=False, num_devices=NCORES)

    # x: per chunk c the stationary block [x_hi | x_lo] (128,64) bf16
    x_d = nc.dram_tensor("x", [128, NCH * 2 * T], BF16, kind="ExternalInput")
    # w: chunk-major; per chunk c: [w_hi (128,512) | w_lo (128,512)]
    w_d = nc.dram_tensor("w", [128, NCH * 2 * K], BF16, kind="ExternalInput")
    # consts: tposG (128,1) f32 = 32 - (p%32); bdiag (128,128) bf16 block-diag;
    # bq f32 (128,4) block cols; bqb bf16 (128,4); bmask bf16 (1,512) block rows
    tp_d = nc.dram_tensor("tposg", [128, 1], F32, kind="ExternalInput")
    bd_d = nc.dram_tensor("bdiag", [128, 128], BF16, kind="ExternalInput")
    bq_d = nc.dram_tensor("bqf", [128, 4], F32, kind="ExternalInput")
    bqb_d = nc.dram_tensor("bqb", [128, 4], BF16, kind="ExternalInput")
    bm_d = nc.dram_tensor("bmask", [1, 512], BF16, kind="ExternalInput")
    out_d = nc.dram_tensor("out", [128, 128], F32, kind="ExternalOutput")

    with TileContext(nc) as tc:
        with (
            tc.tile_pool(name="sb", bufs=1) as sb,
            tc.tile_pool(name="wp", bufs=6) as wp,
            tc.tile_pool(name="ps", bufs=1, space="PSUM") as ps,
            tc.tile_pool(name="dram", bufs=1, space="DRAM") as dr,
        ):
            xsb = sb.tile([128, NCH * 2 * T], BF16)
            # first chunks early (sync queue) so matmul can start; the rest
            # split across both queues ahead of their first weight groups.
            # x rides SWDGE (gpsimd): Q7 is free until the CC bootstrap
            # barrier begins (~21us), keeping the HWDGE queues pure weight
            nc.gpsimd.dma_start(xsb[:, 0:GROUP * 2 * T], x_d[:, 0:GROUP * 2 * T])
            nc.gpsimd.dma_start(xsb[:, GROUP * 2 * T:], x_d[:, GROUP * 2 * T:])

            widths = [b - a for a, b in SPLITS]
            wbase = []
            off = 0
            for w in widths:
                wbase.append(off)
                off += NCH * 2 * w
            acc = [ps.tile([2 * T, w], F32, name=f"acc{s}")
                   for s, w in enumerate(widths)]
            S = sb.tile([128, 128], F32)
            bins = [dr.tile([(b - a) // 4, 128], F32, name=f"bins{s}")
                    for s, (a, b) in enumerate(SPLITS)]
            bouts = [dr.tile([(b - a) // 4, 128], F32, addr_space="Shared",
                             name=f"bouts{s}")
                     for s, (a, b) in enumerate(SPLITS)]

            # feature-major weight stream: split s covers features [a,b); per
            # chunk [w_hi | w_lo] (ws each).  AR0 (48KB) fires while split 1
            # still streams; only AR1 (16KB) is exposed at the end.
            def emit_split(s):
                a, b = SPLITS[s]
                w = widths[s]
                grp = GROUPS[s]
                bounds = list(range(0, NCH, grp)) + [NCH]
                if bounds[-2] == NCH:
                    bounds = bounds[:-1]
                for gi in range(len(bounds) - 1):
                    c0, c1 = bounds[gi], bounds[gi + 1]
                    wt = wp.tile([128, (c1 - c0) * 2 * w], BF16, tag=f"wt{s}")
                    eng = nc.sync if (gi + QPAR[s]) % 2 == 0 else nc.scalar
                    base = wbase[s] + c0 * 2 * w
                    eng.dma_start(wt[:], w_d[:, base:base + (c1 - c0) * 2 * w])
                    for c in range(c0, c1):
                        xst = xsb[:, c * 2 * T:(c + 1) * 2 * T]
                        wof = (c - c0) * 2 * w
                        nc.tensor.matmul(acc[s][:], xst, wt[:, wof:wof + w],
                                         start=(c == 0), stop=False)
                        nc.tensor.matmul(acc[s][:], xst,
                                         wt[:, wof + w:wof + 2 * w],
                                         start=False, stop=(c == NCH - 1))
                # fold split s into packed S rows (feature f=128q+c -> row
                # 32q+t), then AllReduce those rows
                q0, q1 = a // 128, b // 128
                for q in range(q0, q1):
                    dst = S[32 * q:32 * q + 32, :]
                    cof = 128 * q - a
                    nc.scalar.copy(dst, acc[s][0:T, cof:cof + 128])
                    nc.vector.tensor_tensor(dst, dst,
                                            acc[s][T:2 * T, cof:cof + 128],
                                            mybir.AluOpType.add)
                nc.gpsimd.dma_start(bins[s][:], S[32 * q0:32 * q1, :])
                nc.gpsimd.collective_compute(
                    "AllReduce", mybir.AluOpType.add,
                    replica_groups=[list(range(NCORES))],
                    ins=[bins[s].opt()], outs=[bouts[s].opt()])

            for s in range(len(SPLITS)):
                emit_split(s)

            # ---------------- epilogue (every core, packed layout) ----------
            tpos = sb.tile([128, 1], F32)
            nc.sync.dma_start(tpos[:], tp_d[:])
            bdiag = sb.tile([128, 128], BF16)
            nc.sync.dma_start(bdiag[:], bd_d[:])
            bqf = sb.tile([128, 4], F32)
            nc.sync.dma_start(bqf[:], bq_d[:])
            bqb = sb.tile([128, 4], BF16)
            nc.sync.dma_start(bqb[:], bqb_d[:])
            bmask = sb.tile([1, 512], BF16)
            nc.sync.dma_start(bmask[:], bm_d[:])

            # per-split readback + epilogue prework: split-0 parts run while
            # AR1 is still in flight.  spikeG/pvG are zero-initialized so the
            # per-block matmuls (which contract over all 128 partitions with
            # zero weights outside their block) never see garbage.
            G = sb.tile([128, 128], F32)
            spikeG = sb.tile([128, 128], BF16)
            potG = sb.tile([128, 128], F32)
            pvG = sb.tile([128, 128], F32)
            nspkb = ps.tile([128, 128], F32, name="nspkb")
            vals_ps = ps.tile([1, 512], F32, name="vals")
            nspk_ps = ps.tile([1, 512], F32, name="nspkr")
            nc.vector.memset(spikeG[:], 0)
            nc.vector.memset(pvG[:], 0)
            for s, (a, b) in enumerate(SPLITS):
                q0, q1 = a // 128, b // 128  # noqa: prework after all ARs
                eng = nc.sync if s % 2 == 0 else nc.scalar
                eng.dma_start(G[32 * q0:32 * q1, :], bouts[s][:])
                # engine base partitions must be 0/32/64: round the op row
                # range down (overlap rows recompute identical values)
                r0, r1 = min(32 * q0, 64), 32 * q1
                nc.vector.tensor_scalar(spikeG[r0:r1, :], G[r0:r1, :], THRESH,
                                        None, op0=mybir.AluOpType.is_gt)
                nc.vector.scalar_tensor_tensor(potG[r0:r1, :], G[r0:r1, :],
                                               THRESH, G[r0:r1, :],
                                               op0=mybir.AluOpType.is_gt,
                                               op1=mybir.AluOpType.mult)
                # nspk broadcast within each 32-row block of this split
                nc.tensor.matmul(nspkb[r0:r1, :], bdiag[:, r0:r1], spikeG[:],
                                 start=True, stop=True)
                nc.vector.scalar_tensor_tensor(pvG[r0:r1, :], nspkb[r0:r1, :],
                                               tpos[r0:r1, :], potG[r0:r1, :],
                                               op0=mybir.AluOpType.is_equal,
                                               op1=mybir.AluOpType.mult)
                for q in range(q0, q1):
                    nc.tensor.matmul(vals_ps[0:1, 128 * q:128 * (q + 1)],
                                     bqf[:, q:q + 1], pvG[:],
                                     start=True, stop=True)
                    nc.tensor.matmul(nspk_ps[0:1, 128 * q:128 * (q + 1)],
                                     bqb[:, q:q + 1], spikeG[:],
                                     start=True, stop=True)

            # v*32 = max(values) * 32  (values[k] is 0 exactly when nspk==0)
            vmax = sb.tile([1, 1], F32)
            nc.vector.tensor_reduce(vmax[:], vals_ps[:], axis=mybir.AxisListType.X,
                                    op=mybir.AluOpType.max)
            vmax32 = sb.tile([1, 1], F32)
            nc.vector.tensor_scalar(vmax32[:], vmax[:], float(T), None,
                                    op0=mybir.AluOpType.mult)
            nrow = sb.tile([1, 512], F32)
            nc.scalar.copy(nrow[:], nspk_ps[:])
            # total = (values + vmax32) * nspk
            total = sb.tile([1, 512], F32)
            nc.vector.scalar_tensor_tensor(total[:], vals_ps[:], vmax32[:], nrow[:],
                                           op0=mybir.AluOpType.add,
                                           op1=mybir.AluOpType.mult)

            # top-16 nonzero mask: two rounds of (8-max, match-replace-with-0)
            work = sb.tile([1, 512], F32)
            s8a = sb.tile([1, 8], F32)
            nc.vector.max(s8a[:], total[:])
            nc.vector.match_replace(work[:], s8a[:], total[:], 0.0)
            s8b = sb.tile([1, 8], F32)
            nc.vector.max(s8b[:], work[:])
            nc.vector.match_replace(work[:], s8b[:], work[:], 0.0)
            cmask = sb.tile([1, 512], BF16)
            nc.vector.tensor_tensor(cmask[:], total[:], work[:],
                                    mybir.AluOpType.is_gt)

            # coef in packed layout: 4 block-row broadcasts of cmask slices
            coefG = ps.tile([128, 128], F32, name="coefg")
            for q in range(4):
                nc.tensor.matmul(coefG[:], bmask[0:1, 128 * q:128 * (q + 1)],
                                 cmask[0:1, 128 * q:128 * (q + 1)],
                                 start=(q == 0), stop=(q == 3))
            resG = sb.tile([128, 128], F32)
            nc.vector.scalar_tensor_tensor(resG[:], coefG[:], 0.0, spikeG[:],
                                           op0=mybir.AluOpType.is_gt,
                                           op1=mybir.AluOpType.mult)
            nc.sync.dma_start(out_d[:], resG[:])

    nc.compile()
    return nc


def _get_nc():
    if "nc" not in _CACHE:
        _CACHE["nc"] = _build_nc()
    return _CACHE["nc"]


def _split_bf16(a):
    """Split fp32 array into (hi, lo) bf16 parts: hi + lo == a to ~2^-18 rel."""
    hi = a.astype(NPBF16)
    lo = (a - hi.astype(np.float32)).astype(NPBF16)
    return hi, lo


def _consts():
    p = np.arange(128)
    tpos = (float(T) - (p % 32)).astype(np.float32).reshape(128, 1)
    bdiag = ((p[:, None] // 32) == (p[None, :] // 32)).astype(NPBF16)
    bqf = ((p[:, None] // 32) == np.arange(4)[None, :]).astype(np.float32)
    bqb = bqf.astype(NPBF16)
    # bmask[0, 128q + m] = (m//32 == q): stationary (1,128) slices for the
    # coef broadcast (output partition m belongs to block q)
    bm = np.zeros((1, 512), dtype=NPBF16)
    for q in range(4):
        seg = np.zeros(128, np.float32)
        seg[32 * q:32 * (q + 1)] = 1.0
        bm[0, 128 * q:128 * (q + 1)] = seg.astype(NPBF16)
    return tpos, bdiag, bqf, bqb, bm


def _pack_inputs(rec_field, weight):
    X = np.ascontiguousarray(np.asarray(rec_field, dtype=np.float32).reshape(T, CTOT))
    W = np.ascontiguousarray(np.asarray(weight, dtype=np.float32).reshape(K, CTOT))
    tpos, bdiag, bqf, bqb, bm = _consts()
    in_maps = []
    for i in range(NCORES):
        xp = np.zeros((T, SHP), np.float32)
        xp[:, :SH] = X[:, i * SH:(i + 1) * SH]
        wp = np.zeros((K, SHP), np.float32)
        wp[:, :SH] = W[:, i * SH:(i + 1) * SH]
        # (contract, n) -> chunks (NCH,128,n)
        xpc = xp.T.reshape(NCH, 128, T)
        wpc = wp.T.reshape(NCH, 128, K)
        xh, xl = _split_bf16(xpc)
        wh, wl = _split_bf16(wpc)
        # per chunk stationary [x_hi | x_lo]: (NCH,128,2T) -> (128, NCH*2T)
        xpk = np.ascontiguousarray(
            np.concatenate([xh, xl], axis=2).transpose(1, 0, 2).reshape(128, NCH * 2 * T))
        # w split-major, chunk-major within: per split s, chunk c:
        # [w_hi (ws) | w_lo (ws)]
        parts = []
        for a, b in SPLITS:
            blk = np.concatenate([wh[:, :, a:b], wl[:, :, a:b]], axis=2)
            parts.append(blk.transpose(1, 0, 2).reshape(128, -1))
        wpk = np.ascontiguousarray(np.concatenate(parts, axis=1))
        in_maps.append({"x": xpk, "w": wpk, "tposg": tpos, "bdiag": bdiag,
                        "bqf": bqf, "bqb": bqb, "bmask": bm})
    return in_maps


def kernel(rec_field, weight, _trace=False, _trace_kwargs=None):
    nc = _get_nc()
    in_maps = _pack_inputs(rec_field, weight)
    r = run_bass_kernel_spmd(nc, in_maps, list(range(NCORES)), trace=_trace,
                             **(_trace_kwargs or {}))
    _CACHE["last_results"] = r
    res = np.asarray(r.results[0]["out"], dtype=np.float32)  # (128,128) packed
    out = res.reshape(4, 32, 128).transpose(1, 0, 2).reshape(T, K)
    return np.ascontiguousarray(out).reshape(T, K, 1, 1)
